# revision 19
# baseline (speedup 1.0000x reference)
"""Per-token sparse MoE kernel for Trainium2 (8 NeuronCores, Bass/Tile).

Problem: y[b,t,:] = sum_e relu(x[b,t]@gw[t])[e] * (gelu(x[b,t]@W1[t,e]+b1)@W2[t,e]+b2)
Shapes: x[2048,16,128], W1[16,4,128,512], W2[16,4,512,128], gates[16,128,4].

Sharding: the t dimension (16) is split across the 8 cores (2 t-values per
core). That makes the problem embarrassingly parallel (no collectives) and
each core only loads its own 1/8 of the weights (~2.1 MB in bf16) instead
of the full 33 MB, so the kernel is compute-bound rather than HBM-bound.

Host-side marshalling (inside kernel(), part of sharding): inputs are
sliced per-core, cast to the matmul dtype, and x is pre-transposed to
xT[t, d, b] so the device program needs no transpose/cast machinery for
its inputs.

Per-core device dataflow, per t:
  gate_T [E,B] = relu(gw^T @ xT)  (PE, gw stationary; ACT relu w/ bias)
  gate    [B,E]  by PE-transposing gate_T back (per 128-column block)
  h_T [H,B] = W1-slice^T @ xT     (PE, W1 stationary, 16 matmuls N=512)
  h = gelu(h_T + b1)              (ACT, exact-erf Gelu, per-partition bias)
  expert psum [Bblk,D] = h-block^T @ W2-block (PE, 4 accumulating matmuls)
  y += gate[:,e] * psum           (DVE tensor_scalar + batched adds)

b2 is all-zero in this problem; a host-side numpy correction covers the
general case.
"""

import contextlib
import ctypes
import sys
import types

import numpy as np

B, T, D, E, H = 2048, 16, 128, 4, 512
N_CORES = 8
T_LOC = T // N_CORES  # 2 t-values per core
NBLK = B // 128       # 16 b-blocks of 128
NCH = B // 512        # 4 b-chunks of 512 (matmul moving-operand max)

_CACHE: dict = {}


def _install_ntff_hook():
    """Provide antenv.axon_hooks (absent in this image) so that
    run_bass_kernel_spmd(trace=True) can capture NTFF profiles."""
    if "antenv.axon_hooks" in sys.modules:
        return
    try:
        lib = ctypes.CDLL("/opt/axon/libaxon_pjrt.so")
        if not hasattr(lib, "axon_start_nrt_profile"):
            hook = None
        else:
            lib.axon_start_nrt_profile.argtypes = [
                ctypes.POINTER(ctypes.c_int64),
                ctypes.c_size_t,
            ]
            lib.axon_start_nrt_profile.restype = ctypes.c_int64
            lib.axon_stop_nrt_profile.argtypes = [ctypes.c_char_p]
            lib.axon_stop_nrt_profile.restype = ctypes.c_int64

            @contextlib.contextmanager
            def hook(output_dir, device_ids):
                import jax

                jax.devices()
                if device_ids:
                    ids = (ctypes.c_int64 * len(device_ids))(*device_ids)
                    rc = lib.axon_start_nrt_profile(ids, len(device_ids))
                else:
                    rc = lib.axon_start_nrt_profile(None, 0)
                if rc != 0:
                    raise RuntimeError(f"axon_start_nrt_profile rc={rc}")
                try:
                    yield
                finally:
                    lib.axon_stop_nrt_profile(str(output_dir).encode())

        m = types.ModuleType("antenv.axon_hooks")
        m.get_axon_ntff_profile_hook = lambda: hook
        m.set_axon_ntff_profile_hook = lambda h: None
        sys.modules["antenv.axon_hooks"] = m
        import antenv

        antenv.axon_hooks = m
    except OSError:
        pass


def _build(dt_mm_name: str = "bfloat16"):
    """Build and compile the per-core Bass program. Same program on all cores.

    dt_mm_name selects the matmul-operand dtype (host pre-casts inputs):
      float32  — exact, but every matmul is a 2-pass HI/LO pair (slow)
      float32r — single-pass fp22-truncated reads (~2.6e-4 rel err)
      bfloat16 — single-pass + fast weight load (~4e-3 rel err)
    PSUM accumulation is fp32 in all cases.
    """
    import concourse.bass as bass
    import concourse.tile as tile
    from concourse import bacc, mybir

    dt_mm = getattr(mybir.dt, dt_mm_name)
    f32 = mybir.dt.float32
    AF = mybir.ActivationFunctionType

    nc = bacc.Bacc("TRN2", target_bir_lowering=False, debug=False, num_devices=N_CORES)

    xT_d = nc.dram_tensor("xT", [T_LOC, D, B], dt_mm, kind="ExternalInput").ap()
    w1_d = nc.dram_tensor("w1", [T_LOC, E, D, H], dt_mm, kind="ExternalInput").ap()
    b1_d = nc.dram_tensor("b1t", [T_LOC, E, 128, 4], f32, kind="ExternalInput").ap()
    w2_d = nc.dram_tensor("w2", [T_LOC, E, H, D], dt_mm, kind="ExternalInput").ap()
    gw_d = nc.dram_tensor("gw", [T_LOC, D, E], dt_mm, kind="ExternalInput").ap()
    gb_d = nc.dram_tensor("gb", [T_LOC, E], f32, kind="ExternalInput").ap()
    id_d = nc.dram_tensor("ident", [E, E], dt_mm, kind="ExternalInput").ap()
    y_d = nc.dram_tensor("y", [B, T_LOC, D], f32, kind="ExternalOutput").ap()

    with tile.TileContext(nc) as tc, contextlib.ExitStack() as ctx:
        ep = ctx.enter_context
        # SBUF pools
        const_p = ep(tc.tile_pool(name="const", bufs=1))
        xT_p = ep(tc.tile_pool(name="xT", bufs=2))
        h_p = ep(tc.tile_pool(name="h", bufs=3))
        w1_p = ep(tc.tile_pool(name="w1", bufs=3))
        w2_p = ep(tc.tile_pool(name="w2", bufs=3))
        y_p = ep(tc.tile_pool(name="y", bufs=2))
        tmp_p = ep(tc.tile_pool(name="tmp", bufs=4))
        gt_p = ep(tc.tile_pool(name="gt", bufs=2))
        small_p = ep(tc.tile_pool(name="small", bufs=4))
        # PSUM pools: hps 3x2 + sp 2 = 8 banks (gate/transpose psums share "sp")
        hps_p = ep(tc.tile_pool(name="hps", bufs=3, space="PSUM"))
        sps_p = ep(tc.tile_pool(name="sps", bufs=2, space="PSUM"))

        ident = const_p.tile([E, E], dt_mm)
        nc.gpsimd.dma_start(ident[:], id_d[:])

        # ---- software pipeline over (tl, e): mm1 runs one step ahead of
        # mm2 so the gelu (ACT) stream never starves, including across the
        # t boundary.
        xT_t, gate_t, y_t, w1e0_t, b1e0_t = {}, {}, {}, {}, {}

        def emit_t_head(tl):
            w1e0_t[tl] = w1_p.tile([128, H], dt_mm, tag="w1", name=f"w1_{tl}_0")
            nc.sync.dma_start(w1e0_t[tl][:], w1_d[tl, 0])
            b1e0_t[tl] = small_p.tile([128, 4], f32, tag="b1", name=f"b1_{tl}_0")
            nc.sync.dma_start(b1e0_t[tl][:], b1_d[tl, 0])
            xT = xT_p.tile([128, B], dt_mm, tag="xT", name=f"xT_{tl}")
            xT_t[tl] = xT
            for c in range(NCH):
                nc.sync.dma_start(
                    xT[:, 512 * c : 512 * (c + 1)], xT_d[tl, :, 512 * c : 512 * (c + 1)]
                )

        def emit_gate(tl):
            xT = xT_t[tl]
            gw_sb = small_p.tile([128, E], dt_mm, tag="gw", name=f"gw_{tl}")
            nc.gpsimd.dma_start(gw_sb[:], gw_d[tl])
            gb_sb = small_p.tile([E, 1], f32, tag="gb", name=f"gb_{tl}")
            nc.gpsimd.dma_start(gb_sb[:], gb_d[tl])
            gate_T = gt_p.tile([E, B], dt_mm, tag="gateT", name=f"gateT_{tl}")
            for c in range(NCH):
                gps = sps_p.tile([E, 512], f32, tag="sp", name=f"gps_{tl}_{c}")
                nc.tensor.matmul(
                    gps[:], gw_sb[:], xT[:, 512 * c : 512 * (c + 1)],
                    start=True, stop=True,
                )
                nc.vector.tensor_scalar(
                    gate_T[:, 512 * c : 512 * (c + 1)], gps[:],
                    gb_sb[:, 0:1], 0.0,
                    bass.mybir.AluOpType.add, bass.mybir.AluOpType.max,
                )
            # gate [B, E] per-block by PE-transposing gate_T back
            gate_sb = gt_p.tile([128, E * NBLK], f32, tag="gate", name=f"gate_{tl}")
            gate_t[tl] = gate_sb
            for blk in range(NBLK):
                tp = sps_p.tile([128, E], dt_mm, tag="sp", name=f"tp_{tl}_{blk}")
                nc.tensor.transpose(
                    tp[:], gate_T[:, 128 * blk : 128 * (blk + 1)], ident[:]
                )
                nc.vector.tensor_copy(gate_sb[:, E * blk : E * (blk + 1)], tp[:])

        def emit_mm1(tl, e):
            xT = xT_t[tl]
            if e == 0:
                w1_sb, b1_sb = w1e0_t[tl], b1e0_t[tl]
            else:
                w1_sb = w1_p.tile([128, H], dt_mm, tag="w1", name=f"w1_{tl}_{e}")
                nc.sync.dma_start(w1_sb[:], w1_d[tl, e])
                b1_sb = small_p.tile([128, 4], f32, tag="b1", name=f"b1_{tl}_{e}")
                nc.sync.dma_start(b1_sb[:], b1_d[tl, e])
            # h_T = gelu(W1slice^T @ xT + b1), laid out [128, (hb b)]
            h_sb = h_p.tile([128, 4 * B], dt_mm, tag="h", name=f"h_{tl}_{e}")
            for cc in range(2):  # cc-outer: mm2 blocks 0-7 unblock after 4 gelus
                for hb in range(4):
                    hps = hps_p.tile(
                        [128, 1024], f32, tag="hps", name=f"hps_{tl}_{e}_{hb}_{cc}"
                    )
                    for half in range(2):
                        c = 2 * cc + half
                        nc.tensor.matmul(
                            hps[:, 512 * half : 512 * (half + 1)],
                            w1_sb[:, 128 * hb : 128 * (hb + 1)],
                            xT[:, 512 * c : 512 * (c + 1)],
                            start=True, stop=True,
                        )
                    nc.scalar.activation(
                        h_sb[:, B * hb + 1024 * cc : B * hb + 1024 * (cc + 1)],
                        hps[:], AF.Gelu, bias=b1_sb[:, hb : hb + 1],
                    )
            return h_sb

        def emit_mm2(tl, e, h_sb):
            gate_sb, y_sb = gate_t[tl], y_t[tl]
            w2_sb = w2_p.tile([128, H], dt_mm, tag="w2", name=f"w2_{tl}_{e}")
            nc.sync.dma_start(
                w2_sb[:].rearrange("p (hk d) -> p hk d", hk=4),
                w2_d[tl, e].rearrange("(hk p) d -> p hk d", p=128),
            )
            # expert out per 128-block, gated accumulate into y
            for g in range(4):  # groups of 4 blocks -> batched adds
                if e > 0:
                    tmp = tmp_p.tile([128, 512], f32, tag="tmp", name=f"tmp_{tl}_{e}_{g}")
                else:
                    tmp = None
                for j in range(4):
                    blk = 4 * g + j
                    yps = sps_p.tile([128, 128], f32, tag="sp", name=f"yps_{tl}_{e}_{blk}")
                    for hk in range(4):
                        nc.tensor.matmul(
                            yps[:],
                            h_sb[:, B * hk + 128 * blk : B * hk + 128 * (blk + 1)],
                            w2_sb[:, 128 * hk : 128 * (hk + 1)],
                            start=(hk == 0), stop=(hk == 3),
                        )
                    gcol = gate_sb[:, E * blk + e : E * blk + e + 1]
                    if e == 0:
                        nc.vector.tensor_scalar(
                            y_sb[:, 512 * g + 128 * j : 512 * g + 128 * (j + 1)],
                            yps[:], gcol, None, bass.mybir.AluOpType.mult,
                        )
                    else:
                        nc.vector.tensor_scalar(
                            tmp[:, 128 * j : 128 * (j + 1)],
                            yps[:], gcol, None, bass.mybir.AluOpType.mult,
                        )
                if e > 0:
                    nc.gpsimd.tensor_add(
                        y_sb[:, 512 * g : 512 * (g + 1)],
                        y_sb[:, 512 * g : 512 * (g + 1)],
                        tmp[:],
                    )

        def emit_store(tl):
            nc.sync.dma_start(
                y_d[:, tl, :].rearrange("(blk p) d -> p blk d", p=128),
                y_t[tl][:].rearrange("p (blk d) -> p blk d", blk=NBLK),
            )

        # two-ahead pipeline: mm1(k+2) is emitted right after mm2(k), so the
        # gelu (ACT) stream always has a full expert of work buffered.
        seq = [(tl, e) for tl in range(T_LOC) for e in range(E)]
        emit_t_head(0)
        y_t[0] = y_p.tile([128, B], f32, tag="y", name="y_0")
        hs = {}
        hs[seq[0]] = emit_mm1(*seq[0])
        emit_gate(0)
        hs[seq[1]] = emit_mm1(*seq[1])
        for k, (tl, e) in enumerate(seq):
            if (tl, e) == (0, 1):
                emit_t_head(1)  # prefetch t1 inputs well before they're needed
            if (tl, e) == (1, 0):
                y_t[1] = y_p.tile([128, B], f32, tag="y", name="y_1")
                emit_gate(1)
            emit_mm2(tl, e, hs.pop((tl, e)))
            if (tl, e) == (1, 0):
                emit_store(0)
            if k + 2 < len(seq):
                hs[seq[k + 2]] = emit_mm1(*seq[k + 2])
        emit_store(1)

    nc.compile()
    return nc


def get_program(dt_mm_name: str = "bfloat16"):
    key = ("nc", dt_mm_name)
    if key not in _CACHE:
        _install_ntff_hook()
        _CACHE[key] = _build(dt_mm_name)
    return _CACHE[key]


def _np_dt(dt_mm_name):
    if dt_mm_name == "bfloat16":
        import ml_dtypes

        return ml_dtypes.bfloat16
    return np.float32


def make_in_maps(x, W1, b1, W2, b2, gate_w_infer, gate_b_infer, dt_mm_name="bfloat16"):
    c = np.ascontiguousarray
    ndt = _np_dt(dt_mm_name)
    x = np.asarray(x, np.float32)
    W1 = np.asarray(W1, np.float32)
    b1 = np.asarray(b1, np.float32)
    W2 = np.asarray(W2, np.float32)
    gw = np.asarray(gate_w_infer, np.float32)
    gb = np.asarray(gate_b_infer, np.float32)
    ident = np.eye(E, dtype=np.float32)
    maps = []
    for i in range(N_CORES):
        s = slice(T_LOC * i, T_LOC * (i + 1))
        # xT[t, d, b] pre-transposed; b1 as [t, e, h%128, h//128]
        xTi = np.transpose(x[:, s, :], (1, 2, 0))
        b1i = np.transpose(b1[s].reshape(T_LOC, E, 4, 128), (0, 1, 3, 2))
        maps.append(
            {
                "xT": c(xTi.astype(ndt)),
                "w1": c(W1[s].astype(ndt)),
                "b1t": c(b1i),
                "w2": c(W2[s].astype(ndt)),
                "gw": c(gw[s].astype(ndt)),
                "gb": c(gb[s]),
                "ident": ident.astype(ndt),
            }
        )
    return maps


def kernel(x, W1, b1, W2, b2, gate_w_infer, gate_b_infer):
    from concourse.bass_utils import run_bass_kernel_spmd

    dt_mm_name = "bfloat16"
    nc = get_program(dt_mm_name)
    maps = make_in_maps(x, W1, b1, W2, b2, gate_w_infer, gate_b_infer, dt_mm_name)
    res = run_bass_kernel_spmd(nc, maps, list(range(N_CORES)))
    y = np.concatenate([res.results[i]["y"] for i in range(N_CORES)], axis=1)
    b2 = np.asarray(b2, np.float32)
    if np.any(b2):
        # b2 is all-zero for this problem's setup_inputs; handled host-side
        # for generality since the device kernel omits the b2 term.
        xf = np.asarray(x, np.float32)
        gate = np.einsum("btd,tde->bte", xf, np.asarray(gate_w_infer, np.float32))
        gate = np.maximum(gate + np.asarray(gate_b_infer, np.float32), 0.0)
        y = y + np.einsum("bte,ted->btd", gate, b2)
    return y, np.asarray(0.0, dtype=np.float32)


# revision 20
# speedup vs baseline: 1.0085x; 1.0085x over previous
"""Per-token sparse MoE kernel for Trainium2 (8 NeuronCores, Bass/Tile).

Problem: y[b,t,:] = sum_e relu(x[b,t]@gw[t])[e] * (gelu(x[b,t]@W1[t,e]+b1)@W2[t,e]+b2)
Shapes: x[2048,16,128], W1[16,4,128,512], W2[16,4,512,128], gates[16,128,4].

Sharding: the t dimension (16) is split across the 8 cores (2 t-values per
core). That makes the problem embarrassingly parallel (no collectives) and
each core only loads its own 1/8 of the weights (~2.1 MB in bf16) instead
of the full 33 MB, so the kernel is compute-bound rather than HBM-bound.

Host-side marshalling (inside kernel(), part of sharding): inputs are
sliced per-core, cast to the matmul dtype, and x is pre-transposed to
xT[t, d, b] so the device program needs no transpose/cast machinery for
its inputs.

Per-core device dataflow, per t:
  gate_T [E,B] = relu(gw^T @ xT)  (PE, gw stationary; ACT relu w/ bias)
  gate    [B,E]  by PE-transposing gate_T back (per 128-column block)
  h_T [H,B] = W1-slice^T @ xT     (PE, W1 stationary, 16 matmuls N=512)
  h = gelu(h_T + b1)              (ACT, exact-erf Gelu, per-partition bias)
  expert psum [Bblk,D] = h-block^T @ W2-block (PE, 4 accumulating matmuls)
  y += gate[:,e] * psum           (DVE tensor_scalar + batched adds)

b2 is all-zero in this problem; a host-side numpy correction covers the
general case.
"""

import contextlib
import ctypes
import sys
import types

import numpy as np

B, T, D, E, H = 2048, 16, 128, 4, 512
N_CORES = 8
T_LOC = T // N_CORES  # 2 t-values per core
NBLK = B // 128       # 16 b-blocks of 128
NCH = B // 512        # 4 b-chunks of 512 (matmul moving-operand max)

_CACHE: dict = {}


def _install_ntff_hook():
    """Provide antenv.axon_hooks (absent in this image) so that
    run_bass_kernel_spmd(trace=True) can capture NTFF profiles."""
    if "antenv.axon_hooks" in sys.modules:
        return
    try:
        lib = ctypes.CDLL("/opt/axon/libaxon_pjrt.so")
        if not hasattr(lib, "axon_start_nrt_profile"):
            hook = None
        else:
            lib.axon_start_nrt_profile.argtypes = [
                ctypes.POINTER(ctypes.c_int64),
                ctypes.c_size_t,
            ]
            lib.axon_start_nrt_profile.restype = ctypes.c_int64
            lib.axon_stop_nrt_profile.argtypes = [ctypes.c_char_p]
            lib.axon_stop_nrt_profile.restype = ctypes.c_int64

            @contextlib.contextmanager
            def hook(output_dir, device_ids):
                import jax

                jax.devices()
                if device_ids:
                    ids = (ctypes.c_int64 * len(device_ids))(*device_ids)
                    rc = lib.axon_start_nrt_profile(ids, len(device_ids))
                else:
                    rc = lib.axon_start_nrt_profile(None, 0)
                if rc != 0:
                    raise RuntimeError(f"axon_start_nrt_profile rc={rc}")
                try:
                    yield
                finally:
                    lib.axon_stop_nrt_profile(str(output_dir).encode())

        m = types.ModuleType("antenv.axon_hooks")
        m.get_axon_ntff_profile_hook = lambda: hook
        m.set_axon_ntff_profile_hook = lambda h: None
        sys.modules["antenv.axon_hooks"] = m
        import antenv

        antenv.axon_hooks = m
    except OSError:
        pass


def _build(dt_mm_name: str = "bfloat16"):
    """Build and compile the per-core Bass program. Same program on all cores.

    dt_mm_name selects the matmul-operand dtype (host pre-casts inputs):
      float32  — exact, but every matmul is a 2-pass HI/LO pair (slow)
      float32r — single-pass fp22-truncated reads (~2.6e-4 rel err)
      bfloat16 — single-pass + fast weight load (~4e-3 rel err)
    PSUM accumulation is fp32 in all cases.
    """
    import concourse.bass as bass
    import concourse.tile as tile
    from concourse import bacc, mybir

    dt_mm = getattr(mybir.dt, dt_mm_name)
    f32 = mybir.dt.float32
    AF = mybir.ActivationFunctionType

    nc = bacc.Bacc("TRN2", target_bir_lowering=False, debug=False, num_devices=N_CORES)

    xT_d = nc.dram_tensor("xT", [T_LOC, D, B], dt_mm, kind="ExternalInput").ap()
    w1_d = nc.dram_tensor("w1", [T_LOC, E, D, H], dt_mm, kind="ExternalInput").ap()
    b1_d = nc.dram_tensor("b1t", [T_LOC, E, 128, 4], f32, kind="ExternalInput").ap()
    w2_d = nc.dram_tensor("w2", [T_LOC, E, H, D], dt_mm, kind="ExternalInput").ap()
    gw_d = nc.dram_tensor("gw", [T_LOC, D, E], dt_mm, kind="ExternalInput").ap()
    gb_d = nc.dram_tensor("gb", [T_LOC, E], f32, kind="ExternalInput").ap()
    id_d = nc.dram_tensor("ident", [E, E], dt_mm, kind="ExternalInput").ap()
    y_d = nc.dram_tensor("y", [B, T_LOC, D], f32, kind="ExternalOutput").ap()

    with tile.TileContext(nc) as tc, contextlib.ExitStack() as ctx:
        ep = ctx.enter_context
        # SBUF pools
        const_p = ep(tc.tile_pool(name="const", bufs=1))
        xT_p = ep(tc.tile_pool(name="xT", bufs=2))
        h_p = ep(tc.tile_pool(name="h", bufs=3))
        w1_p = ep(tc.tile_pool(name="w1", bufs=3))
        w2_p = ep(tc.tile_pool(name="w2", bufs=3))
        y_p = ep(tc.tile_pool(name="y", bufs=2))
        tmp_p = ep(tc.tile_pool(name="tmp", bufs=4))
        gt_p = ep(tc.tile_pool(name="gt", bufs=2))
        small_p = ep(tc.tile_pool(name="small", bufs=4))
        # PSUM pools: hps 3x2 + sp 2 = 8 banks (gate/transpose psums share "sp")
        hps_p = ep(tc.tile_pool(name="hps", bufs=3, space="PSUM"))
        sps_p = ep(tc.tile_pool(name="sps", bufs=2, space="PSUM"))

        ident = const_p.tile([E, E], dt_mm)
        nc.gpsimd.dma_start(ident[:], id_d[:])

        # ---- software pipeline over (tl, e): mm1 runs one step ahead of
        # mm2 so the gelu (ACT) stream never starves, including across the
        # t boundary.
        xT_t, gate_t, y_t, w1e0_t, b1e0_t = {}, {}, {}, {}, {}

        def emit_t_head(tl):
            w1e0_t[tl] = w1_p.tile([128, H], dt_mm, tag="w1", name=f"w1_{tl}_0")
            nc.sync.dma_start(w1e0_t[tl][:], w1_d[tl, 0])
            b1e0_t[tl] = small_p.tile([128, 4], f32, tag="b1", name=f"b1_{tl}_0")
            nc.sync.dma_start(b1e0_t[tl][:], b1_d[tl, 0])
            xT = xT_p.tile([128, B], dt_mm, tag="xT", name=f"xT_{tl}")
            xT_t[tl] = xT
            for c in range(NCH):
                nc.sync.dma_start(
                    xT[:, 512 * c : 512 * (c + 1)], xT_d[tl, :, 512 * c : 512 * (c + 1)]
                )

        def emit_gate(tl):
            xT = xT_t[tl]
            gw_sb = small_p.tile([128, E], dt_mm, tag="gw", name=f"gw_{tl}")
            nc.gpsimd.dma_start(gw_sb[:], gw_d[tl])
            gb_sb = small_p.tile([E, 1], f32, tag="gb", name=f"gb_{tl}")
            nc.gpsimd.dma_start(gb_sb[:], gb_d[tl])
            gate_T = gt_p.tile([E, B], dt_mm, tag="gateT", name=f"gateT_{tl}")
            for c in range(NCH):
                gps = sps_p.tile([E, 512], f32, tag="sp", name=f"gps_{tl}_{c}")
                nc.tensor.matmul(
                    gps[:], gw_sb[:], xT[:, 512 * c : 512 * (c + 1)],
                    start=True, stop=True,
                )
                nc.vector.tensor_scalar(
                    gate_T[:, 512 * c : 512 * (c + 1)], gps[:],
                    gb_sb[:, 0:1], 0.0,
                    bass.mybir.AluOpType.add, bass.mybir.AluOpType.max,
                )
            # gate [B, E] per-block by PE-transposing gate_T back
            gate_sb = gt_p.tile([128, E * NBLK], f32, tag="gate", name=f"gate_{tl}")
            gate_t[tl] = gate_sb
            for blk in range(NBLK):
                tp = sps_p.tile([128, E], dt_mm, tag="sp", name=f"tp_{tl}_{blk}")
                nc.tensor.transpose(
                    tp[:], gate_T[:, 128 * blk : 128 * (blk + 1)], ident[:]
                )
                nc.vector.tensor_copy(gate_sb[:, E * blk : E * (blk + 1)], tp[:])

        def emit_mm1(tl, e):
            xT = xT_t[tl]
            if e == 0:
                w1_sb, b1_sb = w1e0_t[tl], b1e0_t[tl]
            else:
                w1_sb = w1_p.tile([128, H], dt_mm, tag="w1", name=f"w1_{tl}_{e}")
                nc.sync.dma_start(w1_sb[:], w1_d[tl, e])
                b1_sb = small_p.tile([128, 4], f32, tag="b1", name=f"b1_{tl}_{e}")
                nc.sync.dma_start(b1_sb[:], b1_d[tl, e])
            # h_T = gelu(W1slice^T @ xT + b1), laid out [128, (hb b)]
            h_sb = h_p.tile([128, 4 * B], dt_mm, tag="h", name=f"h_{tl}_{e}")
            for cc in range(2):  # cc-outer: mm2 blocks 0-7 unblock after 4 gelus
                for hb in range(4):
                    hps = hps_p.tile(
                        [128, 1024], f32, tag="hps", name=f"hps_{tl}_{e}_{hb}_{cc}"
                    )
                    for half in range(2):
                        c = 2 * cc + half
                        nc.tensor.matmul(
                            hps[:, 512 * half : 512 * (half + 1)],
                            w1_sb[:, 128 * hb : 128 * (hb + 1)],
                            xT[:, 512 * c : 512 * (c + 1)],
                            start=True, stop=True,
                        )
                    nc.scalar.activation(
                        h_sb[:, B * hb + 1024 * cc : B * hb + 1024 * (cc + 1)],
                        hps[:], AF.Gelu, bias=b1_sb[:, hb : hb + 1],
                    )
            return h_sb

        def emit_mm2(tl, e, h_sb):
            gate_sb, y_sb = gate_t[tl], y_t[tl]
            final = e == E - 1
            w2_sb = w2_p.tile([128, H], dt_mm, tag="w2", name=f"w2_{tl}_{e}")
            nc.sync.dma_start(
                w2_sb[:].rearrange("p (hk d) -> p hk d", hk=4),
                w2_d[tl, e].rearrange("(hk p) d -> p hk d", p=128),
            )
            # expert out per 128-block, gated accumulate into y
            for g in range(4):  # groups of 4 blocks -> batched adds
                if e > 0:
                    tmp = tmp_p.tile([128, 512], f32, tag="tmp", name=f"tmp_{tl}_{e}_{g}")
                else:
                    tmp = None
                for j in range(4):
                    blk = 4 * g + j
                    yps = sps_p.tile([128, 128], f32, tag="sp", name=f"yps_{tl}_{e}_{blk}")
                    for hk in range(4):
                        nc.tensor.matmul(
                            yps[:],
                            h_sb[:, B * hk + 128 * blk : B * hk + 128 * (blk + 1)],
                            w2_sb[:, 128 * hk : 128 * (hk + 1)],
                            start=(hk == 0), stop=(hk == 3),
                        )
                    gcol = gate_sb[:, E * blk + e : E * blk + e + 1]
                    if e == 0:
                        nc.vector.tensor_scalar(
                            y_sb[:, 512 * g + 128 * j : 512 * g + 128 * (j + 1)],
                            yps[:], gcol, None, bass.mybir.AluOpType.mult,
                        )
                    else:
                        nc.vector.tensor_scalar(
                            tmp[:, 128 * j : 128 * (j + 1)],
                            yps[:], gcol, None, bass.mybir.AluOpType.mult,
                        )
                if e > 0:
                    # final expert's adds stay on DVE (faster op) since they
                    # gate the chunked output store
                    add_eng = nc.vector if final else nc.gpsimd
                    add_eng.tensor_add(
                        y_sb[:, 512 * g : 512 * (g + 1)],
                        y_sb[:, 512 * g : 512 * (g + 1)],
                        tmp[:],
                    )
                if final:
                    # store this 512-row chunk of y immediately
                    nc.sync.dma_start(
                        y_d[512 * g : 512 * (g + 1), tl, :].rearrange(
                            "(blk p) d -> p blk d", p=128
                        ),
                        y_sb[:, 512 * g : 512 * (g + 1)].rearrange(
                            "p (blk d) -> p blk d", blk=4
                        ),
                    )

        # two-ahead pipeline: mm1(k+2) is emitted right after mm2(k), so the
        # gelu (ACT) stream always has a full expert of work buffered.
        seq = [(tl, e) for tl in range(T_LOC) for e in range(E)]
        emit_t_head(0)
        y_t[0] = y_p.tile([128, B], f32, tag="y", name="y_0")
        hs = {}
        hs[seq[0]] = emit_mm1(*seq[0])
        emit_gate(0)
        hs[seq[1]] = emit_mm1(*seq[1])
        for k, (tl, e) in enumerate(seq):
            if (tl, e) == (0, 1):
                emit_t_head(1)  # prefetch t1 inputs well before they're needed
            if (tl, e) == (1, 0):
                y_t[1] = y_p.tile([128, B], f32, tag="y", name="y_1")
                emit_gate(1)
            emit_mm2(tl, e, hs.pop((tl, e)))
            if k + 2 < len(seq):
                hs[seq[k + 2]] = emit_mm1(*seq[k + 2])

    nc.compile()
    return nc


def get_program(dt_mm_name: str = "bfloat16"):
    key = ("nc", dt_mm_name)
    if key not in _CACHE:
        _install_ntff_hook()
        _CACHE[key] = _build(dt_mm_name)
    return _CACHE[key]


def _np_dt(dt_mm_name):
    if dt_mm_name == "bfloat16":
        import ml_dtypes

        return ml_dtypes.bfloat16
    return np.float32


def make_in_maps(x, W1, b1, W2, b2, gate_w_infer, gate_b_infer, dt_mm_name="bfloat16"):
    c = np.ascontiguousarray
    ndt = _np_dt(dt_mm_name)
    x = np.asarray(x, np.float32)
    W1 = np.asarray(W1, np.float32)
    b1 = np.asarray(b1, np.float32)
    W2 = np.asarray(W2, np.float32)
    gw = np.asarray(gate_w_infer, np.float32)
    gb = np.asarray(gate_b_infer, np.float32)
    ident = np.eye(E, dtype=np.float32)
    maps = []
    for i in range(N_CORES):
        s = slice(T_LOC * i, T_LOC * (i + 1))
        # xT[t, d, b] pre-transposed; b1 as [t, e, h%128, h//128]
        xTi = np.transpose(x[:, s, :], (1, 2, 0))
        b1i = np.transpose(b1[s].reshape(T_LOC, E, 4, 128), (0, 1, 3, 2))
        maps.append(
            {
                "xT": c(xTi.astype(ndt)),
                "w1": c(W1[s].astype(ndt)),
                "b1t": c(b1i),
                "w2": c(W2[s].astype(ndt)),
                "gw": c(gw[s].astype(ndt)),
                "gb": c(gb[s]),
                "ident": ident.astype(ndt),
            }
        )
    return maps


def kernel(x, W1, b1, W2, b2, gate_w_infer, gate_b_infer):
    from concourse.bass_utils import run_bass_kernel_spmd

    dt_mm_name = "bfloat16"
    nc = get_program(dt_mm_name)
    maps = make_in_maps(x, W1, b1, W2, b2, gate_w_infer, gate_b_infer, dt_mm_name)
    res = run_bass_kernel_spmd(nc, maps, list(range(N_CORES)))
    y = np.concatenate([res.results[i]["y"] for i in range(N_CORES)], axis=1)
    b2 = np.asarray(b2, np.float32)
    if np.any(b2):
        # b2 is all-zero for this problem's setup_inputs; handled host-side
        # for generality since the device kernel omits the b2 term.
        xf = np.asarray(x, np.float32)
        gate = np.einsum("btd,tde->bte", xf, np.asarray(gate_w_infer, np.float32))
        gate = np.maximum(gate + np.asarray(gate_b_infer, np.float32), 0.0)
        y = y + np.einsum("bte,ted->btd", gate, b2)
    return y, np.asarray(0.0, dtype=np.float32)


# revision 21
# speedup vs baseline: 1.0848x; 1.0756x over previous
"""Per-token sparse MoE kernel for Trainium2 (8 NeuronCores, Bass/Tile).

Problem: y[b,t,:] = sum_e relu(x[b,t]@gw[t])[e] * (gelu(x[b,t]@W1[t,e]+b1)@W2[t,e]+b2)
Shapes: x[2048,16,128], W1[16,4,128,512], W2[16,4,512,128], gates[16,128,4].

Sharding: the t dimension (16) is split across the 8 cores (2 t-values per
core). That makes the problem embarrassingly parallel (no collectives) and
each core only loads its own 1/8 of the weights (~2.1 MB in bf16) instead
of the full 33 MB, so the kernel is compute-bound rather than HBM-bound.

Host-side marshalling (inside kernel(), part of sharding): inputs are
sliced per-core, cast to the matmul dtype, and x is pre-transposed to
xT[t, d, b] so the device program needs no transpose/cast machinery for
its inputs.

Per-core device dataflow, per t:
  gate_T [E,B] = relu(gw^T @ xT)  (PE, gw stationary; ACT relu w/ bias)
  gate    [B,E]  by PE-transposing gate_T back (per 128-column block)
  h_T [H,B] = W1-slice^T @ xT     (PE, W1 stationary, 16 matmuls N=512)
  h = gelu(h_T + b1)              (ACT, exact-erf Gelu, per-partition bias)
  expert psum [Bblk,D] = h-block^T @ W2-block (PE, 4 accumulating matmuls)
  y += gate[:,e] * psum           (DVE tensor_scalar + batched adds)

b2 is all-zero in this problem; a host-side numpy correction covers the
general case.
"""

import contextlib
import ctypes
import sys
import types

import numpy as np

B, T, D, E, H = 2048, 16, 128, 4, 512
N_CORES = 8
T_LOC = T // N_CORES  # 2 t-values per core
NBLK = B // 128       # 16 b-blocks of 128
NCH = B // 512        # 4 b-chunks of 512 (matmul moving-operand max)

_CACHE: dict = {}


def _install_ntff_hook():
    """Provide antenv.axon_hooks (absent in this image) so that
    run_bass_kernel_spmd(trace=True) can capture NTFF profiles."""
    if "antenv.axon_hooks" in sys.modules:
        return
    try:
        lib = ctypes.CDLL("/opt/axon/libaxon_pjrt.so")
        if not hasattr(lib, "axon_start_nrt_profile"):
            hook = None
        else:
            lib.axon_start_nrt_profile.argtypes = [
                ctypes.POINTER(ctypes.c_int64),
                ctypes.c_size_t,
            ]
            lib.axon_start_nrt_profile.restype = ctypes.c_int64
            lib.axon_stop_nrt_profile.argtypes = [ctypes.c_char_p]
            lib.axon_stop_nrt_profile.restype = ctypes.c_int64

            @contextlib.contextmanager
            def hook(output_dir, device_ids):
                import jax

                jax.devices()
                if device_ids:
                    ids = (ctypes.c_int64 * len(device_ids))(*device_ids)
                    rc = lib.axon_start_nrt_profile(ids, len(device_ids))
                else:
                    rc = lib.axon_start_nrt_profile(None, 0)
                if rc != 0:
                    raise RuntimeError(f"axon_start_nrt_profile rc={rc}")
                try:
                    yield
                finally:
                    lib.axon_stop_nrt_profile(str(output_dir).encode())

        m = types.ModuleType("antenv.axon_hooks")
        m.get_axon_ntff_profile_hook = lambda: hook
        m.set_axon_ntff_profile_hook = lambda h: None
        sys.modules["antenv.axon_hooks"] = m
        import antenv

        antenv.axon_hooks = m
    except OSError:
        pass


def _build(dt_mm_name: str = "bfloat16"):
    """Build and compile the per-core Bass program. Same program on all cores.

    dt_mm_name selects the matmul-operand dtype (host pre-casts inputs):
      float32  — exact, but every matmul is a 2-pass HI/LO pair (slow)
      float32r — single-pass fp22-truncated reads (~2.6e-4 rel err)
      bfloat16 — single-pass + fast weight load (~4e-3 rel err)
    PSUM accumulation is fp32 in all cases.
    """
    import concourse.bass as bass
    import concourse.tile as tile
    from concourse import bacc, mybir

    dt_mm = getattr(mybir.dt, dt_mm_name)
    f32 = mybir.dt.float32
    AF = mybir.ActivationFunctionType

    nc = bacc.Bacc("TRN2", target_bir_lowering=False, debug=False, num_devices=N_CORES)

    xT_d = nc.dram_tensor("xT", [T_LOC, D, B], dt_mm, kind="ExternalInput").ap()
    w1_d = nc.dram_tensor("w1", [T_LOC, E, D, H], dt_mm, kind="ExternalInput").ap()
    b1_d = nc.dram_tensor("b1t", [T_LOC, E, 128, 4], f32, kind="ExternalInput").ap()
    w2_d = nc.dram_tensor("w2", [T_LOC, E, H, D], dt_mm, kind="ExternalInput").ap()
    gw_d = nc.dram_tensor("gw", [T_LOC, D, E], dt_mm, kind="ExternalInput").ap()
    gb_d = nc.dram_tensor("gb", [T_LOC, E], f32, kind="ExternalInput").ap()
    id_d = nc.dram_tensor("ident", [E, E], dt_mm, kind="ExternalInput").ap()
    y_d = nc.dram_tensor("y", [B, T_LOC, D], f32, kind="ExternalOutput").ap()

    with tile.TileContext(nc) as tc, contextlib.ExitStack() as ctx:
        ep = ctx.enter_context
        # SBUF pools
        const_p = ep(tc.tile_pool(name="const", bufs=1))
        xT_p = ep(tc.tile_pool(name="xT", bufs=2))
        h_p = ep(tc.tile_pool(name="h", bufs=3))
        w1_p = ep(tc.tile_pool(name="w1", bufs=3))
        w2_p = ep(tc.tile_pool(name="w2", bufs=3))
        y_p = ep(tc.tile_pool(name="y", bufs=2))
        tmp_p = ep(tc.tile_pool(name="tmp", bufs=4))
        gt_p = ep(tc.tile_pool(name="gt", bufs=2))
        small_p = ep(tc.tile_pool(name="small", bufs=4))
        # PSUM pools: hps 3x2 + sp 2 = 8 banks (gate/transpose psums share "sp")
        hps_p = ep(tc.tile_pool(name="hps", bufs=3, space="PSUM"))
        sps_p = ep(tc.tile_pool(name="sps", bufs=2, space="PSUM"))

        ident = const_p.tile([E, E], dt_mm)
        nc.gpsimd.dma_start(ident[:], id_d[:])

        # ---- software pipeline over (tl, e): mm1 runs one step ahead of
        # mm2 so the gelu (ACT) stream never starves, including across the
        # t boundary.
        xT_t, gate_t, y_t, w1e0_t, b1e0_t = {}, {}, {}, {}, {}

        def emit_t_head(tl):
            w1e0_t[tl] = w1_p.tile([128, H], dt_mm, tag="w1", name=f"w1_{tl}_0")
            nc.sync.dma_start(w1e0_t[tl][:], w1_d[tl, 0])
            b1e0_t[tl] = small_p.tile([128, 4], f32, tag="b1", name=f"b1_{tl}_0")
            nc.gpsimd.dma_start(b1e0_t[tl][:], b1_d[tl, 0])
            xT = xT_p.tile([128, B], dt_mm, tag="xT", name=f"xT_{tl}")
            xT_t[tl] = xT
            for c in range(NCH):
                nc.sync.dma_start(
                    xT[:, 512 * c : 512 * (c + 1)], xT_d[tl, :, 512 * c : 512 * (c + 1)]
                )

        def emit_gate(tl):
            xT = xT_t[tl]
            gw_sb = small_p.tile([128, E], dt_mm, tag="gw", name=f"gw_{tl}")
            nc.gpsimd.dma_start(gw_sb[:], gw_d[tl])
            gb_sb = small_p.tile([E, 1], f32, tag="gb", name=f"gb_{tl}")
            nc.gpsimd.dma_start(gb_sb[:], gb_d[tl])
            gate_T = gt_p.tile([E, B], dt_mm, tag="gateT", name=f"gateT_{tl}")
            for c in range(NCH):
                gps = sps_p.tile([E, 512], f32, tag="sp", name=f"gps_{tl}_{c}")
                nc.tensor.matmul(
                    gps[:], gw_sb[:], xT[:, 512 * c : 512 * (c + 1)],
                    start=True, stop=True,
                )
                nc.vector.tensor_scalar(
                    gate_T[:, 512 * c : 512 * (c + 1)], gps[:],
                    gb_sb[:, 0:1], 0.0,
                    bass.mybir.AluOpType.add, bass.mybir.AluOpType.max,
                )
            # gate [B, E] per-block by PE-transposing gate_T back
            gate_sb = gt_p.tile([128, E * NBLK], f32, tag="gate", name=f"gate_{tl}")
            gate_t[tl] = gate_sb
            for blk in range(NBLK):
                tp = sps_p.tile([128, E], dt_mm, tag="sp", name=f"tp_{tl}_{blk}")
                nc.tensor.transpose(
                    tp[:], gate_T[:, 128 * blk : 128 * (blk + 1)], ident[:]
                )
                nc.vector.tensor_copy(gate_sb[:, E * blk : E * (blk + 1)], tp[:])

        def emit_mm1(tl, e):
            xT = xT_t[tl]
            if e == 0:
                w1_sb, b1_sb = w1e0_t[tl], b1e0_t[tl]
            else:
                w1_sb = w1_p.tile([128, H], dt_mm, tag="w1", name=f"w1_{tl}_{e}")
                nc.sync.dma_start(w1_sb[:], w1_d[tl, e])
                b1_sb = small_p.tile([128, 4], f32, tag="b1", name=f"b1_{tl}_{e}")
                nc.gpsimd.dma_start(b1_sb[:], b1_d[tl, e])
            # h_T = gelu(W1slice^T @ xT + b1), laid out [128, (hb b)]
            h_sb = h_p.tile([128, 4 * B], dt_mm, tag="h", name=f"h_{tl}_{e}")
            for cc in range(2):  # cc-outer: mm2 blocks 0-7 unblock after 4 gelus
                for hb in range(4):
                    hps = hps_p.tile(
                        [128, 1024], f32, tag="hps", name=f"hps_{tl}_{e}_{hb}_{cc}"
                    )
                    for half in range(2):
                        c = 2 * cc + half
                        nc.tensor.matmul(
                            hps[:, 512 * half : 512 * (half + 1)],
                            w1_sb[:, 128 * hb : 128 * (hb + 1)],
                            xT[:, 512 * c : 512 * (c + 1)],
                            start=True, stop=True,
                        )
                    nc.scalar.activation(
                        h_sb[:, B * hb + 1024 * cc : B * hb + 1024 * (cc + 1)],
                        hps[:], AF.Gelu, bias=b1_sb[:, hb : hb + 1],
                    )
            return h_sb

        def emit_mm2(tl, e, h_sb):
            gate_sb, y_sb = gate_t[tl], y_t[tl]
            final = e == E - 1
            w2_sb = w2_p.tile([128, H], dt_mm, tag="w2", name=f"w2_{tl}_{e}")
            nc.sync.dma_start(
                w2_sb[:].rearrange("p (hk d) -> p hk d", hk=4),
                w2_d[tl, e].rearrange("(hk p) d -> p hk d", p=128),
            )
            # expert out per 128-block, gated accumulate into y
            for g in range(4):  # groups of 4 blocks -> batched adds
                if e > 0:
                    tmp = tmp_p.tile([128, 512], f32, tag="tmp", name=f"tmp_{tl}_{e}_{g}")
                else:
                    tmp = None
                yps = sps_p.tile([128, 512], f32, tag="sp", name=f"yps_{tl}_{e}_{g}")
                for j in range(4):
                    blk = 4 * g + j
                    for hk in range(4):
                        nc.tensor.matmul(
                            yps[:, 128 * j : 128 * (j + 1)],
                            h_sb[:, B * hk + 128 * blk : B * hk + 128 * (blk + 1)],
                            w2_sb[:, 128 * hk : 128 * (hk + 1)],
                            start=(hk == 0), stop=(hk == 3),
                        )
                for j in range(4):
                    blk = 4 * g + j
                    gcol = gate_sb[:, E * blk + e : E * blk + e + 1]
                    if e == 0:
                        nc.vector.tensor_scalar(
                            y_sb[:, 512 * g + 128 * j : 512 * g + 128 * (j + 1)],
                            yps[:, 128 * j : 128 * (j + 1)], gcol, None,
                            bass.mybir.AluOpType.mult,
                        )
                    else:
                        nc.vector.tensor_scalar(
                            tmp[:, 128 * j : 128 * (j + 1)],
                            yps[:, 128 * j : 128 * (j + 1)], gcol, None,
                            bass.mybir.AluOpType.mult,
                        )
                if e > 0:
                    # final expert's adds stay on DVE (faster op) since they
                    # gate the chunked output store
                    add_eng = nc.vector if final else nc.gpsimd
                    add_eng.tensor_add(
                        y_sb[:, 512 * g : 512 * (g + 1)],
                        y_sb[:, 512 * g : 512 * (g + 1)],
                        tmp[:],
                    )
                if final:
                    # store this 512-row chunk of y immediately
                    nc.sync.dma_start(
                        y_d[512 * g : 512 * (g + 1), tl, :].rearrange(
                            "(blk p) d -> p blk d", p=128
                        ),
                        y_sb[:, 512 * g : 512 * (g + 1)].rearrange(
                            "p (blk d) -> p blk d", blk=4
                        ),
                    )

        # two-ahead pipeline: mm1(k+2) is emitted right after mm2(k), so the
        # gelu (ACT) stream always has a full expert of work buffered.
        seq = [(tl, e) for tl in range(T_LOC) for e in range(E)]
        emit_t_head(0)
        y_t[0] = y_p.tile([128, B], f32, tag="y", name="y_0")
        hs = {}
        hs[seq[0]] = emit_mm1(*seq[0])
        emit_gate(0)
        hs[seq[1]] = emit_mm1(*seq[1])
        for k, (tl, e) in enumerate(seq):
            if (tl, e) == (0, 1):
                emit_t_head(1)  # prefetch t1 inputs well before they're needed
            if (tl, e) == (1, 0):
                y_t[1] = y_p.tile([128, B], f32, tag="y", name="y_1")
                emit_gate(1)
            emit_mm2(tl, e, hs.pop((tl, e)))
            if k + 2 < len(seq):
                hs[seq[k + 2]] = emit_mm1(*seq[k + 2])

    nc.compile()
    return nc


def get_program(dt_mm_name: str = "bfloat16"):
    key = ("nc", dt_mm_name)
    if key not in _CACHE:
        _install_ntff_hook()
        _CACHE[key] = _build(dt_mm_name)
    return _CACHE[key]


def _np_dt(dt_mm_name):
    if dt_mm_name == "bfloat16":
        import ml_dtypes

        return ml_dtypes.bfloat16
    return np.float32


def make_in_maps(x, W1, b1, W2, b2, gate_w_infer, gate_b_infer, dt_mm_name="bfloat16"):
    c = np.ascontiguousarray
    ndt = _np_dt(dt_mm_name)
    x = np.asarray(x, np.float32)
    W1 = np.asarray(W1, np.float32)
    b1 = np.asarray(b1, np.float32)
    W2 = np.asarray(W2, np.float32)
    gw = np.asarray(gate_w_infer, np.float32)
    gb = np.asarray(gate_b_infer, np.float32)
    ident = np.eye(E, dtype=np.float32)
    maps = []
    for i in range(N_CORES):
        s = slice(T_LOC * i, T_LOC * (i + 1))
        # xT[t, d, b] pre-transposed; b1 as [t, e, h%128, h//128]
        xTi = np.transpose(x[:, s, :], (1, 2, 0))
        b1i = np.transpose(b1[s].reshape(T_LOC, E, 4, 128), (0, 1, 3, 2))
        maps.append(
            {
                "xT": c(xTi.astype(ndt)),
                "w1": c(W1[s].astype(ndt)),
                "b1t": c(b1i),
                "w2": c(W2[s].astype(ndt)),
                "gw": c(gw[s].astype(ndt)),
                "gb": c(gb[s]),
                "ident": ident.astype(ndt),
            }
        )
    return maps


def kernel(x, W1, b1, W2, b2, gate_w_infer, gate_b_infer):
    from concourse.bass_utils import run_bass_kernel_spmd

    dt_mm_name = "bfloat16"
    nc = get_program(dt_mm_name)
    maps = make_in_maps(x, W1, b1, W2, b2, gate_w_infer, gate_b_infer, dt_mm_name)
    res = run_bass_kernel_spmd(nc, maps, list(range(N_CORES)))
    y = np.concatenate([res.results[i]["y"] for i in range(N_CORES)], axis=1)
    b2 = np.asarray(b2, np.float32)
    if np.any(b2):
        # b2 is all-zero for this problem's setup_inputs; handled host-side
        # for generality since the device kernel omits the b2 term.
        xf = np.asarray(x, np.float32)
        gate = np.einsum("btd,tde->bte", xf, np.asarray(gate_w_infer, np.float32))
        gate = np.maximum(gate + np.asarray(gate_b_infer, np.float32), 0.0)
        y = y + np.einsum("bte,ted->btd", gate, b2)
    return y, np.asarray(0.0, dtype=np.float32)
